# revision 31
# baseline (speedup 1.0000x reference)
"""Trainium2 Bass kernel for the 2-layer LSTM LM (B=8, T=512, H=1024, V=32000).

Self-contained: builds and compiles an SPMD program for 8 NeuronCores on
first call, then executes via run_bass_kernel_spmd (PJRT/axon path).

Sharding: hidden channels 8 ways. Core j owns channels [128j,128j+128) of
every gate and of h/c. The two layer recurrences are software-pipelined
against each other (layer 2 lags layer 1 by D steps) so their serial
chains overlap: per step each core computes its z^T slice [128ch,
4gate x 8batch] with Wh tiles stationary, gate math on 128-partition
tiles, and broadcasts its h slice [128,8] (bf16) to all 8 cores into a
static double-buffered landing slot (descriptors pre-generated one step
ahead; only the doorbell is on the critical path). The xw+b term is
accumulated into PSUM by an identity matmul so ACT reads z directly from
PSUM. Layer-2's input matmul Wi1.h1_t runs on the fly from a small h1
ring archived by the (otherwise idle) SP engine; h2 is archived to a
full sequence for the output projection. The vocab dim of the output
projection is sharded 8 ways (4000 -> padded 4096 out^T rows per core).
Embedding gather, bf16 packing and transposes are host-side prep; the
gather indices/mask specialize the compiled program to the tokens' zero
positions (Keras mask_zero).
"""

import sys

sys.path.insert(0, "/opt/trn_rl_repo")
import numpy as np
import ml_dtypes
import concourse.bass as bass
import concourse.bacc as bacc
import concourse.mybir as mybir
import bass_rust

NC = 8
B = 8
H = 1024
P = 128
KT = H // P     # 8 k-tiles
MT = 4          # gate m-tiles (4 gates x 128ch)
F32 = mybir.dt.float32
BF16 = mybir.dt.bfloat16
F8 = mybir.dt.float8e4
AF = mybir.ActivationFunctionType

XC = 256        # bt-chunk size for xw phase (psum cols)
PC = 512        # bt-chunk size for projection
D = 2           # layer-2 pipeline lag (steps)


def build(T=512, masked_steps=None, n_vt=32, scales=None, verbose=False):
    """scales=(s0, s1): Wh0 is stored as fp8e4 Wh0/s0 (likewise Wi1, Wh1 by
    s1, and h broadcasts in fp8e4); the gate activations multiply z by s_l.
    The L1 xw table holds (x@Wi0 + b0)/s0. scales=None keeps all-bf16."""
    if masked_steps is None:
        masked_steps = {}
    s0, s1 = scales if scales is not None else (1.0, 1.0)
    import os as _os
    WDT = (BF16 if (scales is None or _os.environ.get('FP8_BF16_DEBUG'))
           else F8)
    HDT = BF16 if (WDT == BF16 or _os.environ.get('FP8_WONLY')) else F8
    nm = max(1, len(masked_steps))
    xc = min(XC, T * B)
    pc = min(PC, T * B)
    NXC = T * B // xc          # xw chunks
    NPC = T * B // pc          # proj chunks
    assert T * B % xc == 0 and T * B % pc == 0
    TS_PER_XC = xc // B        # timesteps per xw chunk

    nc = bacc.Bacc(
        "TRN2",
        target_bir_lowering=False,
        debug=False,
        num_devices=NC,
        enable_partition_id=True,
    )

    # ---------------- DRAM ----------------
    xT_d = nc.declare_dram_parameter("xT", [KT, P, T * B], BF16, isOutput=False)
    wi_d = [nc.declare_dram_parameter("wi0", [P, KT * MT * P], BF16, isOutput=False),
            nc.declare_dram_parameter("wi1", [P, KT * MT * P], WDT, isOutput=False)]
    wh_d = [nc.declare_dram_parameter(f"wh{l}", [P, KT * MT * P], WDT, isOutput=False)
            for l in range(2)]
    wo_d = nc.declare_dram_parameter("wo", [P, n_vt * KT * P], BF16, isOutput=False)
    b0_d = nc.declare_dram_parameter("b0", [P, MT], F32, isOutput=False)
    b1r_d = nc.declare_dram_parameter("b1r", [P, MT * B], BF16, isOutput=False)
    idn_d = nc.declare_dram_parameter("idn", [P, P], BF16, isOutput=False)
    bo_d = nc.declare_dram_parameter("bo", [P, n_vt], F32, isOutput=False)
    mt_d = nc.declare_dram_parameter("mtiles", [P, nm * B], F32, isOutput=False)
    out_d = nc.declare_dram_parameter("outT", [n_vt * P, T * B], F32, isOutput=True)

    # ---------------- semaphores ----------------
    dma_in = nc.alloc_semaphore("dma_in")
    xs_sem = [nc.alloc_semaphore(f"xs_sem{i}") for i in range(2)]
    wo_sem = [nc.alloc_semaphore(f"wo_sem{i}") for i in range(2)]
    out_sems = [nc.alloc_semaphore(f"out_sem{i}") for i in range(2)]
    init_sem = nc.alloc_semaphore("init_sem")
    pe_sem = nc.alloc_semaphore("pe_sem")
    act_sem = nc.alloc_semaphore("act_sem")   # ps_big evictions (xw1 + proj)
    prep_sem = nc.alloc_semaphore("prep_sem")
    bar_sem = nc.alloc_semaphore("bar_sem")
    bar_loc = nc.alloc_semaphore("bar_loc")
    recv = [nc.alloc_semaphore(f"recv{l}") for l in range(2)]
    pe_z = [nc.alloc_semaphore(f"pe_z{l}") for l in range(2)]
    act_g = [nc.alloc_semaphore(f"act_g{l}") for l in range(2)]
    dve_c = [nc.alloc_semaphore(f"dve_c{l}") for l in range(2)]
    act_t = [nc.alloc_semaphore(f"act_t{l}") for l in range(2)]
    dve_h = [nc.alloc_semaphore(f"dve_h{l}") for l in range(2)]
    sent = [[nc.alloc_semaphore(f"sent{l}_{i}") for i in range(2)] for l in range(2)]
    arch1 = nc.alloc_semaphore("arch1")
    act_s = [nc.alloc_semaphore(f"act_s{l}") for l in range(2)]

    # ---------------- SBUF ----------------
    # 4-slot broadcast landing buffers (slot = t % 4): a slot written at
    # step t is read by this layer's z at t+1 and by layer-2's xw part at
    # t+D; it is overwritten at t+4, which the PE-order transitive chain
    # (trigger(t+4) <= recv(t+3) <= peers' pe_z <= earlier PE stream)
    # orders after both reads.
    hT2 = nc.alloc_sbuf_tensor("hT2", [P, T * 64], BF16)      # h2 full archive
    hrecv = [nc.alloc_sbuf_tensor(f"hrecv{l}", [P, 4, NC * B], HDT)
             for l in range(2)]
    xw = nc.alloc_sbuf_tensor("xw", [P, MT, T, B], BF16)
    wa = nc.alloc_sbuf_tensor("wa", [P, KT * MT * P], BF16)   # wi0
    wa2 = nc.alloc_sbuf_tensor("wa2", [P, KT * MT * P], WDT)  # wi1
    wb = nc.alloc_sbuf_tensor("wb", [P, KT * MT * P], WDT)    # wh0
    wc = nc.alloc_sbuf_tensor("wc", [P, KT * MT * P], WDT)    # wh1
    xs = nc.alloc_sbuf_tensor("xs", [P, 2, KT, xc], BF16)
    wo_s = nc.alloc_sbuf_tensor("wo_s", [P, 2, KT * P], BF16)
    b0_s = nc.alloc_sbuf_tensor("b0s", [P, MT], F32)
    b1r_s = nc.alloc_sbuf_tensor("b1rs", [P, MT, B], BF16)
    idn_s = nc.alloc_sbuf_tensor("idn_s", [P, P], BF16)
    bo_s = nc.alloc_sbuf_tensor("bo_s", [P, n_vt], F32)
    mt_s = nc.alloc_sbuf_tensor("mt_s", [P, nm * B], F32)
    gt = [nc.alloc_sbuf_tensor(f"gt{l}", [P, MT, B], F32) for l in range(2)]
    ct = [nc.alloc_sbuf_tensor(f"ct{l}", [P, B], F32) for l in range(2)]
    ctm = [nc.alloc_sbuf_tensor(f"ctm{l}", [P, B], F32) for l in range(2)]
    th = [nc.alloc_sbuf_tensor(f"th{l}", [P, B], F32) for l in range(2)]
    tm1 = [nc.alloc_sbuf_tensor(f"tm1_{l}", [P, B], F32) for l in range(2)]
    tm2 = [nc.alloc_sbuf_tensor(f"tm2_{l}", [P, B], F32) for l in range(2)]
    hst = [nc.alloc_sbuf_tensor(f"hst{l}", [P, 2, B], HDT) for l in range(2)]
    scr = nc.alloc_sbuf_tensor("scr", [1, 2], mybir.dt.int32)

    stg = nc.alloc_sbuf_tensor("stg", [P, 2, pc // B, B], F32)
    ps_big = nc.alloc_psum_tensor("ps_big", [P, 2, pc // B, B], F32)
    # one full 2KB psum bank per (layer, parity): bank lq = l*2 + q holds the
    # z accumulation group in its first MT*B elements. Concurrent open groups
    # must not share a 2KB zero region.
    ps_z = nc.alloc_psum_tensor("ps_z", [P, 4, 64, B], F32)

    def ps_z_flat(l, q):
        return bass.AP(ps_z, (l * 2 + q) * 64 * B, [[4 * 64 * B, P], [1, MT * B]])

    whs = [wb, wc]          # recurrent weights per layer
    TP = T + D              # interleaved periods

    blk = nc.Block()
    blk.__enter__()

    def walk(eng):
        """eng in {'SP','PE','ACT','DVE','PL'} - emit that engine's stream.
        All counters are recomputed identically on every pass."""
        PE = nc.tensor
        ACT = nc.scalar
        DVE = nc.vector
        PL = nc.gpsimd
        SP = nc.sync

        c_dma = 0       # dma_in increments
        c_pe = 0        # pe_sem (ps_big matmul groups: xw1 + proj)
        c_big = 0       # ps_big evictions (= act_sem increments)
        c_out = 0       # out_sem increments
        c_prep = 0      # swdge preps (barrier + data broadcasts)

        if eng == "PL":
            r_p8 = PL.to_reg(PL.partition_id() * B)

            def rv_p8():
                # fresh RuntimeValue per use: the value-lowering cache is
                # keyed by object; value is static (own slice offset)
                return bass_rust.make_scalar_value(
                    r_p8, min_val=0, max_val=(NC - 1) * B, guaranteed_mod_val=B)

        # ---- init memsets ----
        if eng == "DVE":
            DVE.memset(hT2[:, :], 0).then_inc(init_sem, 1)
        if eng == "PL":
            PL.memset(hrecv[0][:, :, :], 0)
            PL.memset(hrecv[1][:, :, :], 0)
            PL.memset(ct[0][:, :], 0)
            PL.memset(ct[1][:, :], 0).then_inc(init_sem, 1)
        c_prep += 1
        if eng == "PL":
            # cross-core barrier: no data broadcast may land in a peer's
            # hrecv buffers before that peer zero-initialized them
            PL.wait_ge(init_sem, 2)
            PL.remote_sem_update_broadcast(
                remote_sem=bar_sem,
                local_sem=bar_loc,
                rdests=[(0, kk) for kk in range(NC)],
            ).then_inc(prep_sem, 1)
            PL.wait_ge(prep_sem, c_prep)
            PL.trigger_dma(count=1)
            PL.wait_ge(bar_sem, 16)

        # ---- initial small DMAs (SP) ----
        def din(dst, src):
            nonlocal c_dma
            if eng == "SP":
                SP.dma_start(out=dst, in_=src).then_inc(dma_in, 16)
            c_dma += 16

        din(wa[:, :], wi_d[0][:, :])
        din(wb[:, :], wh_d[0][:, :])
        din(wa2[:, :], wi_d[1][:, :])
        din(wc[:, :], wh_d[1][:, :])
        din(b0_s[:, :], b0_d[:, :])
        din(b1r_s[:, :, :], b1r_d[:, :])
        din(idn_s[:, :], idn_d[:, :])
        din(bo_s[:, :], bo_d[:, :])
        din(mt_s[:, :], mt_d[:, :])
        init_loads = c_dma

        # ================= helpers =================
        def h2chunk(t0, nt, k):
            # [128, nt, 8] slice of hT2 at timestep t0, k-tile k
            return bass.AP(hT2, t0 * 64 + k * 8,
                           [[T * 64, P], [64, nt], [1, B]])

        def xw_phase():
            nonlocal c_dma, c_pe, c_big
            xs_done = {}
            pe_after_chunk = {}
            bias = b0_s[:, :]
            for n in range(NXC):
                if n >= 2 and eng == "SP":
                    SP.wait_ge(pe_sem, pe_after_chunk[n - 2])
                for k in range(KT):
                    if eng == "SP":
                        SP.dma_start(
                            out=xs[:, n % 2, k, :],
                            in_=xT_d[k, :, n * xc : (n + 1) * xc],
                        ).then_inc(xs_sem[n % 2], 16)
                xs_done[n] = 128 * (n // 2 + 1)
                for m in range(MT):
                    bank = (n * MT + m) % 2
                    if eng == "PE":
                        if m == 0:
                            PE.wait_ge(xs_sem[n % 2], xs_done[n])
                        if c_big >= 2:
                            PE.wait_ge(act_sem, c_big - 1)
                    last = None
                    for k in range(KT):
                        if eng == "PE":
                            last = PE.matmul(
                                ps_big[:, bank, 0 : xc // B, :],
                                wa[:, k * 512 + m * P : k * 512 + (m + 1) * P],
                                xs[:, n % 2, k, :],
                                start=(k == 0),
                                stop=(k == KT - 1),
                            )
                    c_pe += 1
                    if eng == "PE":
                        last.then_inc(pe_sem, 1)
                    c_big += 1
                    if eng == "ACT":
                        ACT.wait_ge(pe_sem, c_pe)
                        ACT.activation(
                            xw[:, m, n * TS_PER_XC : (n + 1) * TS_PER_XC, :],
                            ps_big[:, bank, 0 : xc // B, :],
                            AF.Identity,
                            bias=bias[:, m : m + 1],
                            scale=1.0 / s0,
                        ).then_inc(act_sem, 1)
                pe_after_chunk[n] = c_pe

        # ---------- recurrence: one step of layer l at step t ----------
        def rec_pe_xwpart(t2):
            # layer-2 z(t2) accumulation: identity(b1) + Wi1 . h1_{t2}
            q = t2 % 2
            if eng == "PE":
                if t2 == 0:
                    PE.wait_ge(dma_in, init_loads)
                if t2 >= 2:
                    PE.wait_ge(act_g[1], t2 - 1)
                PE.wait_ge(recv[0], 16 * (t2 + 1))
                PE.matmul(
                    ps_z_flat(1, q),
                    idn_s[:, :],
                    b1r_s[:, :, :],
                    start=True, stop=False, skip_group_check=True,
                )
                for k in range(KT):
                    for m in range(MT):
                        PE.matmul(
                            ps_z[:, 2 + q, m, :],
                            wa2[:, k * 512 + m * P : k * 512 + (m + 1) * P],
                            hrecv[0][:, t2 % 4, k * 8 : (k + 1) * 8],
                            start=False, stop=False, skip_group_check=True,
                        )

        def rec_pe_main(l, t):
            # layer-l z(t): (l==0: identity(xw_t)) + Wh_l . h_{t-1}
            q = t % 2
            if eng == "PE":
                if l == 0:
                    if t == 0:
                        PE.wait_ge(dma_in, init_loads)
                        PE.wait_ge(init_sem, 2)
                    if t >= 2:
                        PE.wait_ge(act_g[0], t - 1)
                    PE.wait_ge(act_sem, 4 * (t // TS_PER_XC + 1))
                    PE.matmul(
                        ps_z_flat(0, q),
                        idn_s[:, :],
                        xw[:, :, t, :],
                        start=True, stop=False, skip_group_check=True,
                    )
                if t > 0:
                    PE.wait_ge(recv[l], 16 * t)
                last = None
                for k in range(KT):
                    for m in range(MT):
                        rhs = (hrecv[l][:, 3, k * 8 : (k + 1) * 8] if t == 0
                               else hrecv[l][:, (t - 1) % 4, k * 8 : (k + 1) * 8])
                        last = PE.matmul(
                            ps_z[:, l * 2 + q, m, :],
                            whs[l][:, k * 512 + m * P : k * 512 + (m + 1) * P],
                            rhs,
                            start=False,
                            stop=(k == KT - 1 and m == MT - 1),
                            skip_group_check=True,
                        )
                last.then_inc(pe_z[l], 1)

        def rec_act_gates(l, t):
            q = t % 2
            sl = s0 if l == 0 else s1
            if eng == "ACT":
                ACT.wait_ge(pe_z[l], t + 1)
                ACT.activation(gt[l][:, 0:3, :], ps_z[:, l * 2 + q, 0:3, :],
                               AF.Sigmoid, scale=sl).then_inc(act_s[l], 1)
                ACT.activation(gt[l][:, 3, :], ps_z[:, l * 2 + q, 3, :], AF.Tanh,
                               scale=sl).then_inc(act_g[l], 1)

        def rec_dve_c(l, t):
            mi = masked_steps.get(t)
            if eng == "DVE":
                if t == 0:
                    DVE.wait_ge(act_g[l], t + 1)
                    if mi is None:
                        DVE.tensor_mul(ct[l][:, :], gt[l][:, 0, :], gt[l][:, 3, :]
                                       ).then_inc(dve_c[l], 1)
                    else:
                        mt_ap = mt_s[:, mi * B : (mi + 1) * B]
                        DVE.tensor_mul(ctm[l][:, :], gt[l][:, 0, :], gt[l][:, 3, :])
                        DVE.drain()
                        DVE.tensor_mul(ct[l][:, :], ctm[l][:, :], mt_ap
                                       ).then_inc(dve_c[l], 1)
                elif mi is None:
                    # f*c can start as soon as the sigmoid lands; it overlaps
                    # the g tanh on ACT
                    DVE.wait_ge(act_s[l], t + 1)
                    DVE.tensor_mul(tm2[l][:, :], gt[l][:, 1, :], ct[l][:, :])
                    DVE.wait_ge(act_g[l], t + 1)
                    DVE.tensor_mul(tm1[l][:, :], gt[l][:, 0, :], gt[l][:, 3, :])
                    DVE.drain()
                    DVE.tensor_add(ct[l][:, :], tm1[l][:, :], tm2[l][:, :]
                                   ).then_inc(dve_c[l], 1)
                else:
                    mt_ap = mt_s[:, mi * B : (mi + 1) * B]
                    DVE.wait_ge(act_s[l], t + 1)
                    DVE.tensor_mul(tm2[l][:, :], gt[l][:, 1, :], ct[l][:, :])
                    DVE.wait_ge(act_g[l], t + 1)
                    DVE.tensor_mul(tm1[l][:, :], gt[l][:, 0, :], gt[l][:, 3, :])
                    DVE.drain()
                    DVE.tensor_add(ctm[l][:, :], tm1[l][:, :], tm2[l][:, :])
                    DVE.drain()
                    DVE.select(ct[l][:, :], mt_ap, ctm[l][:, :], ct[l][:, :]
                               ).then_inc(dve_c[l], 1)

        def rec_act_tanh(l, t):
            if eng == "ACT":
                ACT.wait_ge(dve_c[l], t + 1)
                ACT.activation(th[l][:, :], ct[l][:, :], AF.Tanh
                               ).then_inc(act_t[l], 1)

        def rec_dve_h(l, t):
            mi = masked_steps.get(t)
            q = t % 2
            if eng == "DVE":
                DVE.wait_ge(act_t[l], t + 1)
                if t >= 2:
                    DVE.wait_ge(sent[l][q], 16 * (t // 2))
                if mi is None:
                    DVE.tensor_mul(hst[l][:, q, :], gt[l][:, 2, :], th[l][:, :]
                                   ).then_inc(dve_h[l], 1)
                elif t == 0:
                    DVE.tensor_mul(tm1[l][:, :], gt[l][:, 2, :], th[l][:, :])
                    DVE.drain()
                    DVE.tensor_mul(hst[l][:, q, :], tm1[l][:, :],
                                   mt_s[:, mi * B : (mi + 1) * B]
                                   ).then_inc(dve_h[l], 1)
                else:
                    mt_ap = mt_s[:, mi * B : (mi + 1) * B]
                    DVE.tensor_mul(tm1[l][:, :], gt[l][:, 2, :], th[l][:, :])
                    DVE.drain()
                    DVE.select(hst[l][:, q, :], mt_ap, tm1[l][:, :],
                               hst[l][:, (t - 1) % 2, :]).then_inc(dve_h[l], 1)

        def rec_pl_trigger(l, t):
            if eng == "PL":
                PL.wait_ge(prep_sem, c_prep)
                PL.wait_ge(dve_h[l], t + 1)
                if l == 1 and t >= 3:
                    # peers' t+1 broadcast overwrites slot (t+1)%4, which the
                    # hT2 archive copy of step t-3 reads; our trigger gates
                    # their t+1 step
                    PL.wait_ge(arch1, 16 * (t - 2))
                PL.trigger_dma(count=1)

        def rec_pl_descgen(l, t):
            nonlocal c_prep
            c_prep += 1
            if eng == "PL":
                PL.remote_dma_broadcast(
                    out_ap=hrecv[l][:, t % 4, bass.ds(rv_p8(), B)],
                    in_ap=hst[l][:, t % 2, :],
                    remote_sem=recv[l],
                    local_sem=sent[l][t % 2],
                    rdests=[(0, kk) for kk in range(NC)],
                ).then_inc(prep_sem, 1)

        def rec_dve_archive2(t):
            # archive h2_t into the full sequence for the projection
            if eng == "DVE":
                DVE.wait_ge(recv[1], 16 * (t + 1))
                DVE.tensor_scalar_add(hT2[:, t * 64 : (t + 1) * 64],
                                      hrecv[1][:, t % 4, :], 0.0
                                      ).then_inc(arch1, 16)

        def proj_phase():
            nonlocal c_dma, c_pe, c_big, c_out
            wo_done = {}
            pe_after_v = {}
            NT = pc // B  # timesteps per chunk
            pe_base = c_pe
            for v in range(min(2, n_vt)):
                if eng == "SP":
                    SP.dma_start(out=wo_s[:, v % 2, :],
                                 in_=wo_d[:, v * KT * P : (v + 1) * KT * P]
                                 ).then_inc(wo_sem[v % 2], 16)
                wo_done[v] = 16 * (v // 2 + 1)
            for v in range(n_vt):
                for n in range(NPC):
                    g = v * NPC + n  # proj group index
                    bank = g % 2
                    if eng == "PE":
                        if n == 0:
                            PE.wait_ge(wo_sem[v % 2], wo_done[v])
                            if v == 0:
                                PE.wait_ge(arch1, 16 * T)
                        if c_big >= 2:
                            PE.wait_ge(act_sem, c_big - 1)
                        last = None
                        for k in range(KT):
                            last = PE.matmul(
                                ps_big[:, bank, :, :],
                                wo_s[:, v % 2, k * P : (k + 1) * P],
                                h2chunk(n * NT, NT, k),
                                start=(k == 0),
                                stop=(k == KT - 1),
                            )
                        last.then_inc(pe_sem, 1)
                    c_pe += 1
                    c_big += 1
                    if eng == "ACT":
                        ACT.wait_ge(pe_sem, c_pe)
                        if g >= 2:
                            ACT.wait_ge(out_sems[g % 2], 16 * (g // 2))
                        ACT.activation(
                            stg[:, bank, :, :], ps_big[:, bank, :, :],
                            AF.Identity, bias=bo_s[:, v : v + 1],
                        ).then_inc(act_sem, 1)
                    if eng == "SP":
                        SP.wait_ge(act_sem, c_big)
                        SP.dma_start(
                            out=out_d[v * P : (v + 1) * P, n * pc : (n + 1) * pc],
                            in_=stg[:, bank, :, :],
                        ).then_inc(out_sems[g % 2], 16)
                    c_out += 16
                pe_after_v[v] = c_pe
                if v + 2 < n_vt:
                    if eng == "SP":
                        SP.wait_ge(pe_sem, pe_after_v[v])
                        SP.dma_start(out=wo_s[:, (v + 2) % 2, :],
                                     in_=wo_d[:, (v + 2) * KT * P : (v + 3) * KT * P]
                                     ).then_inc(wo_sem[v % 2], 16)
                    wo_done[v + 2] = 16 * ((v + 2) // 2 + 1)

        # ================= main sequence =================
        if eng == "PE":
            PE.wait_ge(dma_in, init_loads)
        xw_phase()

        # bootstrap: desc batch for layer-1 step 0 (fires in period 0)
        rec_pl_descgen(0, 0)

        for p in range(TP):
            t1 = p if p < T else None
            t2 = p - D if p - D >= 0 else None
            # ---- PE ----
            if t2 is not None:
                rec_pe_xwpart(t2)
            if t1 is not None:
                rec_pe_main(0, t1)
            if t2 is not None:
                rec_pe_main(1, t2)
            # ---- ACT / DVE chains ----
            if t1 is not None:
                rec_act_gates(0, t1)
                rec_dve_c(0, t1)
                rec_act_tanh(0, t1)
                rec_dve_h(0, t1)
            if t2 is not None:
                rec_act_gates(1, t2)
                rec_dve_c(1, t2)
                rec_act_tanh(1, t2)
                rec_dve_h(1, t2)
            # ---- PL: triggers (FIFO order), then next-step desc-gens ----
            if t1 is not None:
                rec_pl_trigger(0, t1)
            if t2 is not None:
                rec_pl_trigger(1, t2)
            if t1 is not None and t1 + 1 < T:
                rec_pl_descgen(0, t1 + 1)
            nt2 = p + 1 - D
            if 0 <= nt2 < T:
                rec_pl_descgen(1, nt2)
            # ---- DVE: archive h2 for the projection ----
            if t2 is not None:
                rec_dve_archive2(t2)

        n_pg = n_vt * NPC
        proj_phase()
        if eng == "SP":
            SP.wait_ge(out_sems[0], 16 * ((n_pg + 1) // 2))
            SP.wait_ge(out_sems[1], 16 * (n_pg // 2))
        if eng == "PL":
            # liveness anchor: reg-elimination passes don't see the
            # RegisterAccessPattern read inside the broadcast descs
            PL.reg_save(scr[0:1, 0:1], rv_p8())

    for e in ["SP", "PE", "ACT", "DVE", "PL"]:
        walk(e)

    blk.__exit__(None, None, None)
    _compile_no_dce(nc)
    return nc


def _compile_no_dce(nc):
    """bacc.Bacc.compile() minus dce_regs: the register moves feeding
    RemoteDMA RegisterAccessPatterns are invisible to dce_regs and get
    wrongly eliminated (every descriptor would read offset 0)."""
    nc.insert_bir_kernel_barrier_sem_inc()
    nc.move_matmul_waits_to_ldweights()
    nc.generate_event_semaphores()
    nc.remove_dead_instructions_after_branch()
    nc.validate_blocks()
    nc.thread_jumps()
    nc.remove_dead_blocks()
    nc.remove_dead_allocations()
    nc.verify_switch_hints()
    nc.alloc_regs()
    # inst_simplify dropped: like dce_regs, it cannot see the register reads
    # inside RemoteDMA RegisterAccessPatterns and deletes the register setup
    nc.fuse_regops()
    nc.fuse_blocks()
    nc.replace_nops_with_events()
    for engine in nc.engines:
        nc.fuse_nops(engine)
    nc.remove_dead_nops()
    nc.remove_dangling_data()
    nc.generate_event_semaphores()
    nc.insert_library_loads()
    nc.insert_act_table_loads()
    nc.insert_hostgen_rebases()
    nc.codegen_inst_isa_subclasses()


# ================= host-side packing =================
def pack_inputs(tokens, embed, Wi, Wh, b, Wo, bo, T=512, n_vt=32, fp8=False):
    tokens = np.asarray(tokens)
    embed = np.asarray(embed, dtype=np.float32)
    x = embed[tokens]  # [B, T, H] f32
    xT = np.ascontiguousarray(x.transpose(2, 1, 0)).reshape(KT, P, T * B)
    xT = xT.astype(ml_dtypes.bfloat16)

    if fp8:
        s0 = float(np.abs(Wh[0]).max()) / 240.0
        s1 = float(max(np.abs(Wi[1]).max(), np.abs(Wh[1]).max())) / 240.0
        scales = (s0, s1)
    else:
        s0 = s1 = 1.0
        scales = None

    mask = tokens != 0
    masked_t = [int(t) for t in range(T) if not mask[:, t].all()]
    masked_steps = {t: i for i, t in enumerate(masked_t)}
    nm = max(1, len(masked_t))
    mtiles = np.ones((P, nm * B), np.float32)
    for t, i in masked_steps.items():
        mtiles[:, i * B : (i + 1) * B] = mask[:, t][None, :].astype(np.float32)

    idn = np.eye(P, dtype=ml_dtypes.bfloat16)

    V = Wo.shape[1]
    V8 = V // NC
    in_maps = []
    for j in range(NC):
        cj = np.arange(j * P, (j + 1) * P)
        gate_off = [0, H, 3 * H, 2 * H]  # i, f, o, g
        cols = np.concatenate([off + cj for off in gate_off])

        def pack_w(W, s=None):
            Wj = np.asarray(W, dtype=np.float32)[:, cols]  # [1024, 512]
            t = Wj.reshape(KT, P, MT, P).transpose(1, 0, 2, 3)
            flat = np.ascontiguousarray(t).reshape(P, KT * MT * P)
            if s is None:
                return flat.astype(ml_dtypes.bfloat16)
            import os as _os
            if _os.environ.get('FP8_BF16_DEBUG'):
                return (flat / s).astype(ml_dtypes.bfloat16)
            return (flat / s).astype(ml_dtypes.float8_e4m3fn)

        b0j = (np.asarray(b[0], dtype=np.float32)[cols].reshape(MT, P).T / s0
               ).copy()
        b1j = np.asarray(b[1], dtype=np.float32)[cols].reshape(MT, P).T / s1
        b1r = np.repeat(b1j[:, :, None], B, axis=2).reshape(P, MT * B).astype(
            ml_dtypes.bfloat16)
        woj = np.zeros((H, n_vt * P), np.float32)
        take = min(V8, n_vt * P)
        woj[:, :take] = np.asarray(Wo, dtype=np.float32)[:, j * V8 : j * V8 + take]
        wot = woj.reshape(KT, P, n_vt, P).transpose(1, 2, 0, 3)
        wot = np.ascontiguousarray(wot).reshape(P, n_vt * KT * P).astype(
            ml_dtypes.bfloat16)
        boj = np.zeros((n_vt * P,), np.float32)
        boj[:take] = np.asarray(bo, dtype=np.float32)[j * V8 : j * V8 + take]
        bo_sb = np.ascontiguousarray(boj.reshape(n_vt, P).T)

        sq = None if scales is None else 1.0
        in_maps.append({
            "xT": xT,
            "wi0": pack_w(Wi[0]),
            "wi1": pack_w(Wi[1], s1 if sq else None),
            "wh0": pack_w(Wh[0], s0 if sq else None),
            "wh1": pack_w(Wh[1], s1 if sq else None),
            "wo": wot,
            "b0": np.ascontiguousarray(b0j),
            "b1r": np.ascontiguousarray(b1r),
            "idn": idn,
            "bo": bo_sb,
            "mtiles": mtiles,
        })
    return in_maps, masked_steps, scales


def unpack_outputs(results, T=512, n_vt=32, V=32000):
    V8 = V // NC
    outs = []
    for j in range(NC):
        oT = np.asarray(results[j]["outT"])
        o = oT[:V8].reshape(V8, T, B).transpose(2, 1, 0)
        outs.append(o)
    return np.concatenate(outs, axis=2)


_CACHE = {}
FP8 = False


def _get_compiled(T, masked_key, n_vt, scales=None):
    key = (T, masked_key, n_vt, scales)
    if key not in _CACHE:
        _CACHE[key] = build(T=T, masked_steps=dict(masked_key), n_vt=n_vt,
                            scales=scales)
    return _CACHE[key]


def kernel(tokens, embed, Wi, Wh, b, Wo, bo):
    from concourse.bass_utils import run_bass_kernel_spmd

    tokens = np.asarray(tokens)
    T = tokens.shape[1]
    V = np.asarray(Wo).shape[1]
    n_vt = 32
    in_maps, masked_steps, scales = pack_inputs(tokens, embed, Wi, Wh, b, Wo, bo,
                                                T=T, n_vt=n_vt, fp8=FP8)
    nc = _get_compiled(T, tuple(sorted(masked_steps.items())), n_vt, scales)
    res = run_bass_kernel_spmd(nc, in_maps, core_ids=list(range(NC)))
    out = unpack_outputs(res.results, T=T, n_vt=n_vt, V=V)
    return out.astype(np.float32)
